# revision 8
# baseline (speedup 1.0000x reference)
"""CTRNN policy kernel for Trainium2 (8 NeuronCores, batch-parallel).

Reference computation (per batch element b, B=64, N=1024, OBS=64, A=16):
    I = E[b] @ obs[b]
    repeat 10x:  y = tanh(gain*(v+bias))*mask
                 v = (v + DT/tau * (-v + W[b]@y + I)) * mask
    action[b] = D[b] @ v

Sharding: batch 64 -> 8 cores x 8 individuals, fully data parallel.

Per-core algorithm (algebraic refactor to minimize per-iteration work):
    am = DT/tau*mask, cm = (1-DT/tau)*mask
    Wf = diag(am) @ W @ diag(mask)   (host-folded)
    Ef = diag(am) @ E                (host-folded)
    bc = bias*(1-cm)                 (host)
    state vs = v + bias; then per iteration:
        y   = tanh(g * vs)
        vs' = cm*vs + Wf@y + (Ef@obs + bc)
    finally action = D @ (vs - bias)

Layout: vector index n = p*8 + c maps to SBUF [p=partition(128), c=free(8)].
The matvec Wf@y runs on TensorE as 16 float32r matmuls per iteration:
stationary = y column chunk [128,1], moving = transposed-W slab [128,512],
accumulating into PSUM [1,1024] (row layout, n-ordered), which is then
fused-added with (Ef@obs+bc) on VectorE and DMA-scattered back to [128,8].
"""

import os
import sys
from contextlib import ExitStack

import numpy as np

for _p in ("/opt/trn_rl_repo", "/root/.axon_site/_ro/trn_rl_repo"):
    if os.path.isdir(_p) and _p not in sys.path:
        sys.path.append(_p)

import concourse.bass as bass  # noqa: E402
import concourse.tile as tile  # noqa: E402
from concourse import bacc, mybir  # noqa: E402
from concourse.bass_utils import run_bass_kernel_spmd  # noqa: E402

DT = 0.1
ITERS = int(1.0 // DT)  # == 9: reference.py uses `int(1.0 // DT)`, and 1.0//0.1 == 9.0
B_FULL, N, OBS, ADIM = 64, 1024, 64, 16
NCORES = 8
BPC = B_FULL // NCORES  # individuals per core
P, CN = 128, 8          # n = p*8 + c
F32 = mybir.dt.float32
F32R = mybir.dt.float32r
GROUPS = [(0, 1, 2), (3, 4, 5), (6, 7)]


def kernel_body(ctx, tc, ins, out_ap):
    nc = tc.nc
    Tanh = mybir.ActivationFunctionType.Tanh
    add = mybir.AluOpType.add
    mult = mybir.AluOpType.mult
    sub = mybir.AluOpType.subtract

    const = ctx.enter_context(tc.tile_pool(name="const", bufs=1))
    wpool = ctx.enter_context(tc.tile_pool(name="w", bufs=4))
    etpool = ctx.enter_context(tc.tile_pool(name="et", bufs=2))
    rowpool = ctx.enter_context(tc.tile_pool(name="row", bufs=3))
    scat = ctx.enter_context(tc.tile_pool(name="scat", bufs=6))
    tmp = ctx.enter_context(tc.tile_pool(name="tmp", bufs=6))
    pspool = ctx.enter_context(tc.tile_pool(name="ps", bufs=3, space="PSUM"))
    psact = ctx.enter_context(tc.tile_pool(name="psa", bufs=1, space="PSUM"))

    # ---- constants / persistent state ----
    obs_sb = const.tile([OBS, BPC], F32, tag="obs", name="obs")
    nc.sync.dma_start(obs_sb[:], ins["obsT"][:])
    # per-individual [1, N] row tiles at partition 0 (engine ops need aligned
    # start partitions); seeded with bc, then += Ef@obs on device
    ifb_sb = {}
    for b in range(BPC):
        ifb_sb[b] = const.tile([1, N], F32, tag=f"ifb{b}", name=f"ifb{b}")
        nc.sync.dma_start(ifb_sb[b][:], ins["bc"][b])
    dtr_sb = const.tile([P, BPC * CN * ADIM], F32, tag="dtr", name="dtr")  # [128, 1024]
    for b in range(BPC):
        nc.sync.dma_start(dtr_sb[:, b * CN * ADIM:(b + 1) * CN * ADIM], ins["DTr"][b])

    cm_sb, g_sb, bias_sb, vs_sb, y_sb = {}, {}, {}, {}, {}
    for b in range(BPC):
        cm_sb[b] = const.tile([P, CN], F32, tag=f"cm{b}", name=f"cm{b}")
        nc.sync.dma_start(cm_sb[b][:], ins["cm"][b])
        g_sb[b] = const.tile([P, CN], F32, tag=f"g{b}", name=f"g{b}")
        nc.sync.dma_start(g_sb[b][:], ins["g"][b])
        bias_sb[b] = const.tile([P, CN], F32, tag=f"bias{b}", name=f"bias{b}")
        nc.sync.dma_start(bias_sb[b][:], ins["biasS"][b])
        vs_sb[b] = const.tile([P, CN], F32, tag=f"vs{b}", name=f"vs{b}")
        nc.sync.dma_start(vs_sb[b][:], ins["vs0"][b])
        y_sb[b] = const.tile([P, CN], F32R, tag=f"y{b}", name=f"y{b}")

    act_sb = const.tile([1, BPC * ADIM], F32, tag="act", name="act")

    # ---- W loads (slot-limited by pool bufs; scheduler orders them) ----
    w_sb = {}
    for b in range(BPC):
        w_sb[b] = wpool.tile([P, CN * N], F32R, tag="w", name="w")
        nc.sync.dma_start(w_sb[b][:], ins["Wf"][b])

    # ---- per-individual setup: input current + initial y ----
    for b in range(BPC):
        et = etpool.tile([OBS, N], F32, tag="et", name="et")
        nc.sync.dma_start(et[:], ins["ET"][b])
        ip = pspool.tile([1, N], F32, tag="ps", name="ps")
        for h in range(2):
            nc.tensor.matmul(
                ip[0:1, h * 512:(h + 1) * 512],
                obs_sb[:, b:b + 1],
                et[:, h * 512:(h + 1) * 512],
                start=True, stop=True,
            )
        # Ifb[b] = (Ef@obs) + bc[b]   (in-place: tile was seeded with bc)
        nc.vector.tensor_tensor(ifb_sb[b][:], ip[0:1, :], ifb_sb[b][:], op=add)
        # y0 = tanh(g * vs0)
        t2 = tmp.tile([P, CN], F32, tag="t2", name="t2")
        nc.vector.tensor_tensor(t2[:], g_sb[b][:], vs_sb[b][:], op=mult)
        nc.scalar.activation(y_sb[b][:], t2[:], Tanh)

    # ---- recurrent loop: groups of individuals interleaved per iteration ----
    for group in GROUPS:
        for t in range(ITERS):
            for b in group:
                wy = pspool.tile([1, N], F32, tag="ps", name="ps")
                for c in range(CN):
                    yc = y_sb[b][:, c:c + 1]
                    for h in range(2):
                        nc.tensor.matmul(
                            wy[0:1, h * 512:(h + 1) * 512],
                            yc,
                            w_sb[b][:, c * N + h * 512: c * N + h * 512 + 512],
                            start=(c == 0), stop=(c == CN - 1),
                        )
                u_row = rowpool.tile([1, N], F32, tag="urow", name="urow")
                nc.vector.tensor_tensor(u_row[:], wy[0:1, :], ifb_sb[b][:], op=add)
                u = scat.tile([P, CN], F32, tag="u", name="u")
                nc.sync.dma_start(u[:], u_row[:])  # [1,1024] -> [128,8], n = p*8+c
                t1 = tmp.tile([P, CN], F32, tag="t1", name="t1")
                nc.vector.tensor_tensor(t1[:], cm_sb[b][:], vs_sb[b][:], op=mult)
                nc.vector.tensor_tensor(vs_sb[b][:], t1[:], u[:], op=add)
                if t < ITERS - 1:
                    t2 = tmp.tile([P, CN], F32, tag="t2", name="t2")
                    nc.vector.tensor_tensor(t2[:], g_sb[b][:], vs_sb[b][:], op=mult)
                    nc.scalar.activation(y_sb[b][:], t2[:], Tanh)

    # ---- decode: action = D @ (vs - bias) ----
    for b in range(BPC):
        vf = tmp.tile([P, CN], F32, tag="vf", name="vf")
        nc.vector.tensor_tensor(vf[:], vs_sb[b][:], bias_sb[b][:], op=sub)
        ap = psact.tile([1, ADIM], F32, tag="psa", name="psa")
        for c in range(CN):
            nc.tensor.matmul(
                ap[0:1, :],
                vf[:, c:c + 1],
                dtr_sb[:, b * CN * ADIM + c * ADIM: b * CN * ADIM + (c + 1) * ADIM],
                start=(c == 0), stop=(c == CN - 1),
            )
        nc.vector.tensor_copy(act_sb[0:1, b * ADIM:(b + 1) * ADIM], ap[0:1, :])
    nc.sync.dma_start(out_ap[:], act_sb[0:1, :])


def build_nc():
    nc = bacc.Bacc(
        "TRN2", target_bir_lowering=False, debug=False, enable_asserts=False,
    )
    ins = {}
    for name, shape in [
        ("ET", [BPC, OBS, N]),
        ("DTr", [BPC, P, CN * ADIM]),
        ("obsT", [OBS, BPC]),
        ("vs0", [BPC, P, CN]),
        ("cm", [BPC, P, CN]),
        ("g", [BPC, P, CN]),
        ("biasS", [BPC, P, CN]),
        ("bc", [BPC, N]),
    ]:
        ins[name] = nc.dram_tensor(name, shape, F32, kind="ExternalInput").ap()
    ins["Wf"] = nc.dram_tensor("Wf", [BPC, P, CN * N], F32R, kind="ExternalInput").ap()
    out_ap = nc.dram_tensor("act", [BPC, ADIM], F32, kind="ExternalOutput").ap()

    with tile.TileContext(nc) as tc:
        with ExitStack() as ctx:
            kernel_body(ctx, tc, ins, out_ap)
    nc.compile()
    return nc


def _round_tf32(x):
    """Round fp32 array to tf32 (10-bit mantissa), round-to-nearest-even."""
    u = x.view(np.uint32)
    u = u + (0x0FFF + ((u >> 13) & 1))
    u &= np.uint32(0xFFFFE000)
    return u.view(np.float32)


def prep_in_maps(obs, v0, tau, gain, bias, W, mask, E, D):
    f = np.float32
    obs, v0, tau, gain, bias, W, mask, E, D = [
        np.asarray(x, dtype=f) for x in (obs, v0, tau, gain, bias, W, mask, E, D)
    ]
    am = (DT / tau) * mask                    # [64, N]
    cm = (1.0 - DT / tau) * mask
    Wf = W * am[:, :, None] * mask[:, None, :]
    WT = np.ascontiguousarray(Wf.transpose(0, 2, 1)).reshape(B_FULL, P, CN * N)
    WT = _round_tf32(WT)
    ETp = np.ascontiguousarray((E * am[:, :, None]).transpose(0, 2, 1))  # [64, OBS, N]
    DTp = np.ascontiguousarray(D.transpose(0, 2, 1)).reshape(B_FULL, P, CN * ADIM)
    obsT = np.ascontiguousarray(obs.T)        # [OBS, 64]
    vs0 = np.ascontiguousarray((v0 + bias).reshape(B_FULL, P, CN))
    cmS = np.ascontiguousarray(cm.reshape(B_FULL, P, CN))
    gS = np.ascontiguousarray(gain.reshape(B_FULL, P, CN))
    bS = np.ascontiguousarray(bias.reshape(B_FULL, P, CN))
    bc = np.ascontiguousarray(bias * (1.0 - cm))  # [64, N]

    in_maps = []
    for core in range(NCORES):
        s = slice(core * BPC, (core + 1) * BPC)
        in_maps.append({
            "Wf": np.ascontiguousarray(WT[s]),
            "ET": np.ascontiguousarray(ETp[s]),
            "DTr": np.ascontiguousarray(DTp[s]),
            "obsT": np.ascontiguousarray(obsT[:, s]),
            "vs0": vs0[s], "cm": cmS[s], "g": gS[s], "biasS": bS[s],
            "bc": bc[s],
        })
    return in_maps


_NC_CACHE = None


def _get_nc():
    global _NC_CACHE
    if _NC_CACHE is None:
        _NC_CACHE = build_nc()
    return _NC_CACHE


def kernel(obs, v0, tau, gain, bias, W, mask, E, D):
    nc = _get_nc()
    in_maps = prep_in_maps(obs, v0, tau, gain, bias, W, mask, E, D)
    res = run_bass_kernel_spmd(nc, in_maps, core_ids=list(range(NCORES)))
    return np.concatenate([res.results[c]["act"] for c in range(NCORES)], axis=0)
